# revision 53
# baseline (speedup 1.0000x reference)
"""Trainium2 Bass kernel for nn_Decoder (GNN message passing):
LSTM(1 step) -> GCNConv -> ReLU -> GCNConv -> Linear -> ReLU on a
100K-node / 1.6M-edge graph, SPMD across 8 NeuronCores.

Strategy (dst-node sharding, v2):
- Core c owns nodes [c*12500, (c+1)*12500) and all edges into them.
- Per-node compute (LSTM, x@W transforms) runs feature-major [128, nodes].
- The transformed node table is replicated across cores via per-quarter
  AllGathers into 4 DRAM chunk tables (<=32768 rows each: int16 gather
  index limit); quarters are 128-block-aligned so the AG input is a
  contiguous slice of the node-major bounce buffer.
- GCN propagate: per dst 128-block, gather source rows (bf16) from the
  chunk tables with gpsimd.dma_gather (in-order per-chunk piece streams,
  prefetched), scatter-add via PE matmul against 128x128 selection
  matrices, accumulated in PSUM.
- The gather is descriptor-count-bound on the SWDGE path: 4 SWDGE queues
  (max) with round-robin piece assignment give ~2.2x over one queue.
  single_packet=True wedges the device - do not use.
- Self-loop edges are not gathered: their diag(dinv^2) @ (x W) term is one
  extra PE matmul per block against the node-major mm stage buffer.
- Selection matrices are built in bulk, one gather piece at a time, with
  two broadcast tensor_tensor ops: st = (iota == dstv) * nrm.
- Per-block interleaving: x@W2 runs inside edge layer 1 (quarters AG as
  soon as ready); the final Linear runs inside edge layer 2 with grouped
  output DMAs.
"""

import os
from contextlib import ExitStack

import numpy as np
import ml_dtypes

ABLATE = os.environ.get("KERNEL_ABLATE", "")  # "", "sel1", "sel0", "nogather"
SINGLE_PACKET = os.environ.get("KERNEL_SP", "0") == "1"
NQUEUES = int(os.environ.get("KERNEL_NQ", "4"))
ELEM2X = os.environ.get("KERNEL_E2", "0") == "1"  # 512B elem probe (wrong data)
QRR = os.environ.get("KERNEL_QRR", "1") == "1"  # round-robin queues per piece

import concourse.bacc as bacc
import concourse.mybir as mybir
import concourse.tile as tile
from concourse.bass_utils import run_bass_kernel_spmd

P = 128
N = 100000
NCORES = 8
CH = 4              # src quarters (chunk tables)
G = 4               # dst blocks per run group (tile-padding granularity)
LSTM_CHUNK = 500    # nodes per LSTM column chunk
GBO = 10            # dst blocks per output DMA group

bf16 = ml_dtypes.bfloat16
f32 = np.float32


def _cfg(n=N, ncores=NCORES):
    npc = n // ncores
    nblk = (npc + P - 1) // P
    qb = (nblk + CH - 1) // CH
    qb0 = [min(nblk, q * qb) for q in range(CH + 1)]   # quarter block starts
    q0 = [min(npc, b * P) for b in qb0]                # quarter node starts
    qs = [q0[q + 1] - q0[q] for q in range(CH)]        # quarter sizes (nodes)
    assert all(0 < ncores * s <= 32768 for s in qs)
    return dict(n=n, ncores=ncores, npc=npc, nblk=nblk, qb0=qb0, q0=q0, qs=qs)


# ---------------------------------------------------------------- host prep


def _prep_edges(edge_index, cfg):
    """Sort/pad each core's incident edges into a cross-core-uniform tile
    schedule. Returns per-core device arrays + the static schedule."""
    n, ncores, npc, nblk = cfg["n"], cfg["ncores"], cfg["npc"], cfg["nblk"]
    q0, qs = cfg["q0"], cfg["qs"]

    src = np.asarray(edge_index[0], dtype=np.int64)
    dst = np.asarray(edge_index[1], dtype=np.int64)

    # symmetric norm with self-loops in the degree; self-loop edges are NOT
    # gathered — their diagonal contribution dinv^2*(xW) is added via a
    # per-block diag matmul on the node-major mm stage
    deg = (np.bincount(dst, minlength=n) + 1).astype(np.float64)
    dinv = 1.0 / np.sqrt(deg)
    norm = (dinv[src] * dinv[dst]).astype(np.float32)

    dinv2 = np.zeros((ncores, P, nblk), np.float32)
    d2 = (dinv * dinv).astype(np.float32)
    for c in range(ncores):
        loc = d2[c * npc : (c + 1) * npc]
        pad = np.zeros(nblk * P, np.float32)
        pad[: len(loc)] = loc
        dinv2[c] = pad.reshape(nblk, P).T

    # chunk (src quarter) + row in that chunk's table
    s_rank = src // npc
    s_local = src % npc
    s_ch = np.searchsorted(np.asarray(q0[1 : CH + 1]), s_local, side="right")
    qs_a = np.asarray(qs)
    q0_a = np.asarray(q0[:CH])
    s_row = s_rank * qs_a[s_ch] + (s_local - q0_a[s_ch])

    # runs = (chunk, group of G dst blocks): tiles are 128-edge windows of the
    # run's dst-sorted edge stream; a tile whose edges span several sub-blocks
    # gets one matmul INSTANCE per sub-block (selection shifted by j*128, the
    # is_equal masks foreign edges automatically). Cuts tile padding ~12%.
    ngrp = (nblk + G - 1) // G
    core_of = dst // npc
    per_core = []
    counts = np.zeros((ncores, CH, ngrp), np.int64)
    for c in range(ncores):
        m = core_of == c
        row, d, w, ch = s_row[m], dst[m] - c * npc, norm[m], s_ch[m]
        o = np.lexsort((d, ch))
        row, d, w, ch = row[o], d[o], w[o], ch[o]
        g = (d // P) // G
        counts[c] = np.bincount(ch * ngrp + g, minlength=CH * ngrp).reshape(
            CH, ngrp
        )
        per_core.append((row, d, w, ch, g))

    T_run = (counts.max(axis=0) + P - 1) // P          # [CH, ngrp]
    flat = T_run.reshape(-1)
    base = np.zeros(CH * ngrp + 1, np.int64)
    np.cumsum(flat, out=base[1:])                      # tile offset per run
    TT = int(base[-1])
    NIDX = TT * P

    # per-core fill + instance discovery: (gid, t, j) present for ANY core
    MAXT = int(T_run.max())
    present = np.zeros((CH * ngrp, MAXT, G), bool)
    fills = []
    for c in range(ncores):
        row, d, w, ch, g = per_core[c]
        gid = ch * ngrp + g
        cnt = counts[c].reshape(-1)
        gstart = np.concatenate([[0], np.cumsum(cnt)[:-1]])
        within = np.arange(len(row)) - gstart[gid]
        pos = base[gid] * P + within

        # pads are trailing within each run (= one gather call): negative
        # indices at the end are skipped by the gather ucode (no descriptors)
        idxs = np.full(NIDX, -1, np.int16)
        idxs[pos] = row.astype(np.int16)
        drel = np.full(NIDX, -1, np.int64)              # dst rel to group
        drel[pos] = d - g * (G * P)
        nrmv = np.zeros(NIDX, np.float32)
        nrmv[pos] = w
        j = (d // P) % G
        present[gid, within // P, j] = True
        fills.append((idxs, drel, nrmv))

    # instance table, ordered (chunk, run, tile, j) -> contiguous per run
    inst_col = {}
    inst_tile = []
    inst_shift = []
    run_ib0 = np.zeros(CH * ngrp + 1, np.int64)
    for chn in range(CH):
        for g in range(ngrp):
            gid = chn * ngrp + g
            run_ib0[gid] = len(inst_tile)
            for t in range(int(T_run[chn][g])):
                for j in range(G):
                    if present[gid, t, j]:
                        inst_col[(gid, t, j)] = len(inst_tile)
                        inst_tile.append(int(base[gid]) + t)
                        inst_shift.append(j * P)
    run_ib0[-1] = len(inst_tile)
    NINST = len(inst_tile)
    inst_tile = np.asarray(inst_tile)
    inst_shift = np.asarray(inst_shift)

    arrs = []
    for c in range(ncores):
        idxs, drel, nrmv = fills[c]
        if ELEM2X:
            idxs = idxs // 2  # keep 512B-row probe reads in bounds
        idx16 = np.tile(np.ascontiguousarray(idxs.reshape(-1, 16).T), (8, 1))
        drel_t = drel.reshape(TT, P)                    # [tile, slot]
        nrm_t = nrmv.reshape(TT, P)
        dsti = (drel_t[inst_tile] - inst_shift[:, None]).astype(np.float32)
        nrmi = nrm_t[inst_tile]
        dstt = np.ascontiguousarray(dsti.T).astype(bf16)
        nrmt = np.ascontiguousarray(nrmi.T).astype(bf16)
        gcnt = counts[c].reshape(1, CH * ngrp).astype(np.int32)
        arrs.append((idx16, dstt, nrmt, dinv2[c], gcnt))

    # pieces = runs: per chunk, list of (pt0, pnt, ib0, ni)
    pieces = []
    for chn in range(CH):
        pl = []
        for g in range(ngrp):
            gid = chn * ngrp + g
            pl.append(
                (
                    int(base[gid]),
                    int(T_run[chn][g]),
                    int(run_ib0[gid]),
                    int(run_ib0[gid + 1] - run_ib0[gid]),
                )
            )
        pieces.append(pl)
    MAXI = max(p[3] for pl in pieces for p in pl)

    # per-block instance consumption lists: (chunk, t_local, icol)
    blk_insts = []
    for b in range(nblk):
        g, j = b // G, b % G
        il = []
        for chn in range(CH):
            gid = chn * ngrp + g
            for t in range(int(T_run[chn][g])):
                ic = inst_col.get((gid, t, j))
                if ic is not None:
                    il.append((chn, t, ic))
        blk_insts.append(il)

    sched = dict(
        TT=TT, NIDX=NIDX, NINST=NINST, MAXT=MAXT, MAXI=MAXI,
        pieces=pieces, blk_insts=blk_insts, ngrp=ngrp,
    )
    return arrs, sched


# ---------------------------------------------------------------- device


def _build_nc(cfg, sched):
    ncores, npc, nblk = cfg["ncores"], cfg["npc"], cfg["nblk"]
    qb0, q0, qs = cfg["qb0"], cfg["q0"], cfg["qs"]
    TT, NIDX, NINST, MAXT, MAXI, pieces = (
        sched["TT"],
        sched["NIDX"],
        sched["NINST"],
        sched["MAXT"],
        sched["MAXI"],
        sched["pieces"],
    )
    dt = mybir.dt
    alu = mybir.AluOpType
    act = mybir.ActivationFunctionType

    nc = bacc.Bacc(
        "TRN2",
        target_bir_lowering=False,
        debug=False,
        num_devices=ncores,
        num_swdge_queues=NQUEUES,
    )

    # ---- I/O
    zT_d = nc.dram_tensor("zT", [P, npc], dt.bfloat16, kind="ExternalInput")
    idx_d = nc.dram_tensor("idx16", [P, NIDX // 16], dt.int16, kind="ExternalInput")
    dst_d = nc.dram_tensor("dstv", [P, NINST], dt.bfloat16, kind="ExternalInput")
    nrm_d = nc.dram_tensor("nrmv", [P, NINST], dt.bfloat16, kind="ExternalInput")
    iota_d = nc.dram_tensor("iota", [P, P], dt.bfloat16, kind="ExternalInput")
    wih_d = {
        g: nc.dram_tensor(f"wih_{g}", [P, P], dt.bfloat16, kind="ExternalInput")
        for g in "igo"
    }
    bg_d = {
        g: nc.dram_tensor(f"bg_{g}", [P, 1], dt.float32, kind="ExternalInput")
        for g in "igo"
    }
    w1_d = nc.dram_tensor("w1", [P, P], dt.bfloat16, kind="ExternalInput")
    w2_d = nc.dram_tensor("w2", [P, P], dt.bfloat16, kind="ExternalInput")
    w3t_d = nc.dram_tensor("w3t", [P, P], dt.bfloat16, kind="ExternalInput")
    b1_d = nc.dram_tensor("b1", [P, 1], dt.float32, kind="ExternalInput")
    b2_d = nc.dram_tensor("b2", [P, 1], dt.float32, kind="ExternalInput")
    b3_d = nc.dram_tensor("b3", [P, 1], dt.float32, kind="ExternalInput")
    dinv2_d = nc.dram_tensor("dinv2", [P, nblk], dt.float32, kind="ExternalInput")
    pidx_d = nc.dram_tensor("pidx", [P, 1], dt.float32, kind="ExternalInput")
    ngrp = sched["ngrp"]
    gcnt_d = nc.dram_tensor("gcnt", [1, CH * ngrp], dt.int32, kind="ExternalInput")
    out_d = nc.dram_tensor("outT", [P, npc], dt.float32, kind="ExternalOutput")

    bounce = [nc.dram_tensor(f"bounce{l}", [npc, P], dt.bfloat16) for l in range(2)]
    table = [
        [
            nc.dram_tensor(f"table{l}_{q}", [ncores * qs[q], P], dt.bfloat16)
            for q in range(CH)
        ]
        for l in range(2)
    ]

    with tile.TileContext(nc) as tc, ExitStack() as ctx:
        konst = ctx.enter_context(tc.tile_pool(name="konst", bufs=1))
        xpool = ctx.enter_context(tc.tile_pool(name="xpool", bufs=1))
        spool = ctx.enter_context(tc.tile_pool(name="spool", bufs=1))

        def load_const(handle, shape, dtype):
            t = konst.tile(shape, dtype, tag=handle.name)
            nc.sync.dma_start(t[:], handle[:])
            return t

        iota_t = load_const(iota_d, [P, P], dt.bfloat16)
        wih_t = {g: load_const(wih_d[g], [P, P], dt.bfloat16) for g in "igo"}
        bg_t = {g: load_const(bg_d[g], [P, 1], dt.float32) for g in "igo"}
        w1_t = load_const(w1_d, [P, P], dt.bfloat16)
        w2_t = load_const(w2_d, [P, P], dt.bfloat16)
        w3t_t = load_const(w3t_d, [P, P], dt.bfloat16)
        b1_t = load_const(b1_d, [P, 1], dt.float32)
        b2_t = load_const(b2_d, [P, 1], dt.float32)
        b3_t = load_const(b3_d, [P, 1], dt.float32)
        dinv2_t = load_const(dinv2_d, [P, nblk], dt.float32)
        pidx_t = load_const(pidx_d, [P, 1], dt.float32)
        gcnt_t = load_const(gcnt_d, [1, CH * ngrp], dt.int32)
        gcnt_reg = nc.alloc_register(mybir.EngineType.Pool, "gcnt_reg")
        idx_t = load_const(idx_d, [P, NIDX // 16], dt.int16)
        dst_t = load_const(dst_d, [P, NINST], dt.bfloat16)
        nrm_t = load_const(nrm_d, [P, NINST], dt.bfloat16)

        xT_t = xpool.tile([P, npc], dt.bfloat16, tag="xT")   # x1T then x2T
        stage = spool.tile([P, nblk * P], dt.bfloat16, tag="stage")

        # per-chunk gather-piece pools (double-buffered, in-order streams);
        # created after the LSTM scratch pools close, to fit SBUF
        stgp = [None] * CH
        selp = [None] * CH
        issued = [[-1] * CH for _ in range(2)]      # last piece issued, per layer
        ptiles = [[{} for _ in range(CH)] for _ in range(2)]
        qctr = [0]

        def issue_piece(l, chn):
            k = issued[l][chn] + 1
            if k >= len(pieces[chn]):
                return
            pt0, pnt, ib0, ni = pieces[chn][k]
            qctr[0] += 1
            gid = chn * ngrp + k
            nc.gpsimd.reg_load(gcnt_reg, gcnt_t[0:1, gid : gid + 1])
            stg = stgp[chn].tile([P, MAXT, P], dt.bfloat16, tag=f"stg{chn}")
            nc.gpsimd.dma_gather(
                stg[:, :pnt, :],
                table[l][chn][:],
                idx_t[:, pt0 * 8 : (pt0 + pnt) * 8],
                pnt * P,
                gcnt_reg,
                P,
                single_packet=SINGLE_PACKET,
                queue_num=(qctr[0] % NQUEUES) if QRR else (chn % NQUEUES),
            )
            st = selp[chn].tile([P, MAXI * P], dt.bfloat16, tag=f"sel{chn}")
            stv = st[:, : ni * P].rearrange("p (t c) -> p t c", c=P)
            iotaB = iota_t[:].unsqueeze(1).broadcast_to([P, ni, P])
            dstvB = dst_t[:, ib0 : ib0 + ni].unsqueeze(2).broadcast_to([P, ni, P])
            nrmB = nrm_t[:, ib0 : ib0 + ni].unsqueeze(2).broadcast_to([P, ni, P])
            nc.vector.tensor_tensor(stv, iotaB, dstvB, op=alu.is_equal)
            nc.vector.tensor_tensor(stv, stv, nrmB, op=alu.mult)
            ptiles[l][chn][k] = (stg, st, ib0)
            issued[l][chn] = k

        def quarter_flush(l, q):
            """DMA stage quarter q -> bounce[l], then AllGather the quarter."""
            r0, r1 = q0[q], q0[q + 1]
            full = ((r1 - r0) // P) * P
            if full:
                nc.sync.dma_start(
                    bounce[l][r0 : r0 + full, :].rearrange("(b p) f -> p b f", p=P),
                    stage[:, r0 : r0 + full].rearrange("p (b f) -> p b f", f=P),
                )
            rem = (r1 - r0) - full
            if rem:
                nc.sync.dma_start(
                    bounce[l][r0 + full : r1, :],
                    stage[:rem, r0 + full : r0 + full + P],
                )
            nc.gpsimd.collective_compute(
                "AllGather",
                mybir.AluOpType.bypass,
                replica_groups=[list(range(ncores))],
                ins=[bounce[l][r0:r1, :]],
                outs=[table[l][q][:]],
            )

        # ---- phase 1+2 per quarter: LSTM -> hT, m1 = h @ W1 -> bounce0 -> AG
        # (quarter q's AllGather overlaps later quarters' LSTM/mm work)
        hpool = ctx.enter_context(tc.tile_pool(name="h_pool", bufs=1))
        hT_t = hpool.tile([P, npc], dt.bfloat16, tag="hT")
        with (
            tc.tile_pool(name="lstm_sb", bufs=1) as lsb,
            tc.tile_pool(name="lstm_ps", bufs=6, space="PSUM") as lps,
            tc.tile_pool(name="lstm_tr", bufs=8) as ltr,
            tc.tile_pool(name="mm1_ps", bufs=2, space="PSUM") as mps1,
        ):
            zT_t = lsb.tile([P, npc], dt.bfloat16, tag="zT")
            nc.sync.dma_start(zT_t[:], zT_d[:])

            for q in range(CH):
                c0 = q0[q]
                while c0 < q0[q + 1]:
                    c1 = min(q0[q + 1], c0 + LSTM_CHUNK)
                    w = c1 - c0
                    gate = {}
                    for g in "igo":
                        ps = lps.tile([P, LSTM_CHUNK], dt.float32, tag="ps")
                        nc.tensor.matmul(
                            ps[:, :w], wih_t[g][:], zT_t[:, c0:c1],
                            start=True, stop=True,
                        )
                        fn = act.Tanh if g == "g" else act.Sigmoid
                        sg = ltr.tile([P, LSTM_CHUNK], dt.bfloat16, tag="sg" + g)
                        nc.scalar.activation(
                            sg[:, :w], ps[:, :w], fn, bias=bg_t[g][:]
                        )
                        gate[g] = sg
                    ct = ltr.tile([P, LSTM_CHUNK], dt.bfloat16, tag="ct")
                    nc.vector.tensor_tensor(
                        ct[:, :w], gate["i"][:, :w], gate["g"][:, :w], op=alu.mult
                    )
                    th = ltr.tile([P, LSTM_CHUNK], dt.bfloat16, tag="th")
                    nc.scalar.activation(th[:, :w], ct[:, :w], act.Tanh)
                    nc.vector.tensor_tensor(
                        hT_t[:, c0:c1], gate["o"][:, :w], th[:, :w], op=alu.mult
                    )
                    c0 = c1
                for b in range(qb0[q], qb0[q + 1]):
                    nb = min(P, npc - b * P)
                    pm = mps1.tile([P, P], dt.float32, tag="pm1")
                    nc.tensor.matmul(
                        pm[:nb, :],
                        hT_t[:, b * P : b * P + nb],
                        w1_t[:],
                        start=True,
                        stop=True,
                    )
                    nc.scalar.activation(
                        stage[:nb, b * P : (b + 1) * P], pm[:nb, :], act.Copy
                    )
                quarter_flush(0, q)

        # gather-piece pools (created after the LSTM scratch pools close).
        # Zero both rotating buffers once: slots of skipped (negative) gather
        # indices stay stale, and stale NaN bit patterns would poison the PE
        # MAC even against a zero selection column.
        for chn in range(CH):
            stgp[chn] = ctx.enter_context(tc.tile_pool(name=f"stg{chn}", bufs=2))
            selp[chn] = ctx.enter_context(tc.tile_pool(name=f"sel{chn}", bufs=2))
            for _ in range(2):
                zt = stgp[chn].tile([P, MAXT, P], dt.bfloat16, tag=f"stg{chn}")
                nc.vector.memset(zt[:], 0)

        # -------- phase 2: m1 = h @ W1 per quarter -> bounce0 -> AG
        with (
            tc.tile_pool(name="mm_ps", bufs=2, space="PSUM") as mmps,
            tc.tile_pool(name="agg_ps", bufs=6, space="PSUM") as aps,
            tc.tile_pool(name="dg_pool", bufs=2) as dgp,
        ):

            def self_term(b, nb, pa, stop0):
                # self-loop diagonal: pa += (stage_blk).T-style matmul with
                # diag(dinv^2) built from iota==partition-index
                dg = dgp.tile([P, P], dt.bfloat16, tag="dg")
                nc.vector.tensor_scalar(
                    dg[:nb, :],
                    iota_t[:nb, :],
                    pidx_t[:nb, :],
                    dinv2_t[:nb, b : b + 1],
                    op0=alu.is_equal,
                    op1=alu.mult,
                )
                nc.tensor.matmul(
                    pa[:],
                    stage[:nb, b * P : (b + 1) * P],
                    dg[:nb, :],
                    start=True,
                    stop=stop0,
                )
            for q in range(CH):
                issue_piece(0, q)

            # -------- phase 3+4: edge layer 1 (+ mm2 + AG1 interleaved)
            def post1(b, nb, pa):
                nc.scalar.activation(
                    xT_t[:, b * P : b * P + nb], pa[:, :nb], act.Relu, bias=b1_t[:]
                )
                pm = mmps.tile([P, P], dt.float32, tag="pm")
                nc.tensor.matmul(
                    pm[:nb, :],
                    xT_t[:, b * P : b * P + nb],
                    w2_t[:],
                    start=True,
                    stop=True,
                )
                nc.scalar.activation(
                    stage[:nb, b * P : (b + 1) * P], pm[:nb, :], act.Copy
                )
                for q in range(CH):
                    if b == qb0[q + 1] - 1:
                        quarter_flush(1, q)

            _edge_phase(nc, cfg, sched, 0, ptiles, issue_piece, aps, self_term, post1)

            # -------- phase 5+6: edge layer 2 (+ final Linear interleaved)
            with tc.tile_pool(name="ostage", bufs=2) as opool:
                ost = [None]

                def post2(b, nb, pa):
                    nc.vector.tensor_scalar(
                        xT_t[:, b * P : b * P + nb], pa[:, :nb], b2_t[:], None,
                        op0=alu.add,
                    )
                    g0 = (b // GBO) * GBO
                    if b == g0:
                        ot = opool.tile([P, GBO * P], dt.float32, tag="ost")
                        ost[0] = ot
                    ps = mmps.tile([P, P], dt.float32, tag="pm")
                    nc.tensor.matmul(
                        ps[:, :nb],
                        w3t_t[:],
                        xT_t[:, b * P : b * P + nb],
                        start=True,
                        stop=True,
                    )
                    nc.scalar.activation(
                        ost[0][:, (b - g0) * P : (b - g0) * P + nb],
                        ps[:, :nb],
                        act.Relu,
                        bias=b3_t[:],
                    )
                    if b == min(g0 + GBO, nblk) - 1:
                        c0 = g0 * P
                        c1 = min(npc, (g0 + GBO) * P)
                        nc.sync.dma_start(
                            out_d[:, c0:c1], ost[0][:, : c1 - c0]
                        )

                for q in range(CH):
                    issue_piece(1, q)
                _edge_phase(
                    nc, cfg, sched, 1, ptiles, issue_piece, aps, self_term, post2
                )

    nc.compile()
    return nc


def _edge_phase(nc, cfg, sched, l, ptiles, issue_piece, aps, pre, post):
    npc, nblk = cfg["npc"], cfg["nblk"]
    dt = mybir.dt
    blk_insts = sched["blk_insts"]
    consumed = [-1] * CH
    for b in range(nblk):
        nb = min(P, npc - b * P)
        g = b // G
        pa = aps.tile([P, P], dt.float32, tag="pa")
        il = blk_insts[b]
        pre(b, nb, pa, len(il) == 0)  # self-loop diag matmul (start=True)
        for done, (chn, t, icol) in enumerate(il):
            if g > consumed[chn]:
                consumed[chn] = g
                issue_piece(l, chn)       # prefetch next run's piece
            stg, st, ib0 = ptiles[l][chn][g]
            rc = icol - ib0
            nc.tensor.matmul(
                pa[:],
                stg[:, t, :],
                st[:, rc * P : (rc + 1) * P],
                start=False,
                stop=(done == len(il) - 1),
            )
        post(b, nb, pa)


# ---------------------------------------------------------------- entry


def build(z, edge_index, W_ih, W_hh, b_ih, b_hh, W1, b1, W2, b2, W3, b3, cfg=None):
    """Host prep + trace + compile. Returns (nc, in_maps)."""
    if cfg is None:
        cfg = _cfg()
    ncores, npc = cfg["ncores"], cfg["npc"]
    z = np.asarray(z, dtype=np.float32)
    W_ih = np.asarray(W_ih, dtype=np.float32)
    b = np.asarray(b_ih, dtype=np.float32) + np.asarray(b_hh, dtype=np.float32)

    arrs, sched = _prep_edges(edge_index, cfg)
    nc = _build_nc(cfg, sched)

    gi = {"i": 0, "g": 2, "o": 3}  # torch gate order i,f,g,o (f unused: c0=0)
    common = {
        "iota": np.ascontiguousarray(
            np.tile(np.arange(P, dtype=np.float32), (P, 1))
        ).astype(bf16),
        "w1": np.asarray(W1, np.float32).astype(bf16),
        "w2": np.asarray(W2, np.float32).astype(bf16),
        "w3t": np.ascontiguousarray(np.asarray(W3, np.float32).T).astype(bf16),
        "b1": np.asarray(b1, np.float32).reshape(P, 1).copy(),
        "b2": np.asarray(b2, np.float32).reshape(P, 1).copy(),
        "b3": np.asarray(b3, np.float32).reshape(P, 1).copy(),
        "pidx": np.arange(P, dtype=np.float32).reshape(P, 1).copy(),
    }
    for g, k in gi.items():
        common[f"wih_{g}"] = np.ascontiguousarray(
            W_ih[k * P : (k + 1) * P, :].T
        ).astype(bf16)
        common[f"bg_{g}"] = b[k * P : (k + 1) * P].reshape(P, 1).copy()

    in_maps = []
    for c in range(ncores):
        idx16, dstt, nrmt, d2c, gcnt = arrs[c]
        m = dict(common)
        m["zT"] = np.ascontiguousarray(z[c * npc : (c + 1) * npc].T).astype(bf16)
        m["idx16"] = idx16
        m["dstv"] = dstt
        m["nrmv"] = nrmt
        m["dinv2"] = np.ascontiguousarray(d2c)
        m["gcnt"] = gcnt
        in_maps.append(m)
    return nc, in_maps


def assemble(results, cfg=None):
    if cfg is None:
        cfg = _cfg()
    ncores, npc = cfg["ncores"], cfg["npc"]
    out = np.empty((ncores * npc, P), np.float32)
    for c in range(ncores):
        out[c * npc : (c + 1) * npc] = results[c]["outT"].T
    return out


def kernel(z, edge_index, W_ih, W_hh, b_ih, b_hh, W1, b1, W2, b2, W3, b3):
    nc, in_maps = build(z, edge_index, W_ih, W_hh, b_ih, b_hh, W1, b1, W2, b2, W3, b3)
    res = run_bass_kernel_spmd(nc, in_maps, core_ids=list(range(NCORES)))
    return assemble(res.results)
